# revision 26
# baseline (speedup 1.0000x reference)
"""Trainium2 Bass kernel for nn_MiniLLM (B=4, S=2048, D=8, H=2, HD=4, F=24, V=20).

Key insight: the reference only uses the LAST token's hidden state for the
output logits, so the S^2 attention matrix is dead code.  We need q only at
position S-1 (causal mask there admits all keys), and k/v over all B*S rows.

Layouts (per core):
  G layout: [128, 512] f32 = 16 groups x 8 dims on partitions, 512 rows free.
      partition p = g*8 + d,  g in [0,16);  free f in [0,512)
      flat row = (g//2)*1024 + 2*f + (g%2)      (b = g//4, s = row % 2048)
  R layout: [128 part, 64 t, 8 d] with t = c*16 + g (c in [0,4)), related to
      G by PE transpose of 128x128 chunks: h_R[p, c*16+g, d] = h_G[g*8+d, c*128+p]
      => s(p,c,g) = ((g%4)//2)*1024 + 256*c + 2*p + (g%2)

Embedding gather: gpsimd indirect_copy shares one index per 16-partition
group, so we gather PAIRS of rows with combined index x0*20+x1 from a
[128, 400] product table embT2rep[p, v1*20+v2] = emb[v1 or v2, p%8]
(parity = (p%16)//8 picks v1/v2) - landing h directly in G layout.
"""

import os
import sys
import numpy as np

sys.path.insert(0, "/opt/trn_rl_repo")

import concourse.bass as bass
import concourse.bacc as bacc
from concourse import mybir
from concourse.tile import TileContext
from concourse.bass_utils import run_bass_kernel_spmd

F32 = mybir.dt.float32
F32R = mybir.dt.float32r
U16 = mybir.dt.uint16
AF = mybir.ActivationFunctionType
ALU = mybir.AluOpType
AX = mybir.AxisListType

B, S, D, H, HD, Fdim, V = 4, 2048, 8, 2, 4, 24, 20
EPS = 1e-6
NCORES = 8


def _ap(sliced, free_dims, extra_off=0):
    """Keep the partition dim of a sliced AP, replace the free dims."""
    return bass.AP(
        tensor=sliced.tensor,
        offset=sliced.offset + extra_off,
        ap=[list(sliced.ap[0])] + [list(d) for d in free_dims],
    )


# ----------------------------------------------------------------------------
# Host-side constant tables (shape-derived only; no input data involved)
# ----------------------------------------------------------------------------

def _host_consts():
    c = {}
    # s-position per R coordinate (p, t) ; t = cc*16 + g ; f-direction reversed
    # within each 128-chunk so that s = S-1 lands on partition 0
    p = np.arange(128)[:, None]
    t = np.arange(64)[None, :]
    cc, g = t // 16, t % 16
    s = ((g % 4) // 2) * 1024 + 2 * (128 * cc + 127 - p) + (g % 2)  # [128, 64]
    freqs = 1.0 / (10000.0 ** (np.arange(0, HD, 2) / HD))  # [2] = [1.0, 0.01]
    ang = s[:, :, None].astype(np.float64) * freqs[None, None, :]
    c["cos_t"] = np.cos(ang).astype(np.float32)  # [128, 64, 2]
    c["sin_t"] = np.sin(ang).astype(np.float32)

    ii = np.arange(128)
    c["id128"] = np.eye(128, dtype=np.float32)
    # P0[d, p] = 1 iff p%16 == d ; P1: p%16 == 8+d   (embT2rep builders)
    c["p0"] = (ii[None, :] % 16 == np.arange(8)[:, None]).astype(np.float32)
    c["p1"] = (ii[None, :] % 16 == 8 + np.arange(8)[:, None]).astype(np.float32)
    # replicators E*[d, m] = 1 iff m % D == d
    c["e16"] = (np.arange(128)[None, :] % 8 == np.arange(8)[:, None]).astype(np.float32)
    c["e4"] = (np.arange(32)[None, :] % 8 == np.arange(8)[:, None]).astype(np.float32)
    c["e24"] = (np.arange(96)[None, :] % 24 == np.arange(24)[:, None]).astype(np.float32)
    # block-diag masks
    c["mask16"] = (np.arange(128)[None, :] // 8 == ii[:, None] // 8).astype(np.float32)
    c["m432"] = (np.arange(32)[None, :] // 8 == np.arange(32)[:, None] // 8).astype(np.float32)
    c["m496"] = (np.arange(96)[None, :] // 24 == np.arange(32)[:, None] // 8).astype(np.float32)
    c["m9632"] = (np.arange(32)[None, :] // 8 == np.arange(96)[:, None] // 24).astype(np.float32)
    c["m480"] = (np.arange(80)[None, :] // 20 == np.arange(32)[:, None] // 8).astype(np.float32)
    # expanders
    c["x832"] = (np.arange(32)[None, :] // 4 == np.arange(8)[:, None]).astype(np.float32)
    c["x432"] = (np.arange(32)[None, :] // 8 == np.arange(4)[:, None]).astype(np.float32)
    c["x480"] = (np.arange(80)[None, :] // 20 == np.arange(4)[:, None]).astype(np.float32)
    # bdone4[p, m] = 1 iff m == p//8   (per-batch sum over 8 dims)
    c["bdone4"] = (np.arange(4)[None, :] == np.arange(32)[:, None] // 8).astype(np.float32)
    c["ones_m"] = np.ones((1, 128), np.float32)
    c["one11"] = np.ones((1, 1), np.float32)
    c["ones_c"] = np.ones((128, 1), np.float32)
    c["eps_c"] = np.full((128, 1), EPS, np.float32)
    return c


# packed layouts: name -> (rows, cols); weights are filled per-call
PACKA_CONST = ["mask16", "cos_t", "sin_t", "m432", "m496", "m9632", "m480",
               "x832", "x432", "x480", "bdone4", "e16", "e4", "e24",
               "ones_m", "one11", "ones_c", "eps_c"]
PACKA_WEIGHT = {"g1": (8, 1), "g2": (8, 1), "gf": (8, 1),
                "wq": (8, 8), "wk": (8, 8), "wv": (8, 8), "wo": (8, 8),
                "wup": (8, 24), "wdown": (24, 8), "wvocab": (8, 20)}
PACKR_CONST = ["id128", "p0", "p1"]


def _pack_layout():
    c = _host_consts()
    offs, cur = {}, 0
    for nm in PACKA_CONST:
        a = c[nm].reshape(c[nm].shape[0], -1)
        offs[nm] = (cur, a.shape[0], a.shape[1])
        cur += a.shape[1]
    for nm, (r, w) in PACKA_WEIGHT.items():
        offs[nm] = (cur, r, w)
        cur += w
    na = cur
    offsr, cur = {}, 0
    for nm in PACKR_CONST:
        a = c[nm].reshape(c[nm].shape[0], -1)
        offsr[nm] = (cur, a.shape[0], a.shape[1])
        cur += a.shape[1]
    offsr["emb"] = (cur, V, D)
    cur += D
    return offs, na, offsr, cur


PACKA_OFFS, PACKA_N, PACKR_OFFS, PACKR_N = _pack_layout()


def _build_packs(emb, g1, g2, gf, Wq, Wk, Wv, Wo, Wup, Wdown, Wvocab):
    c = _host_consts()
    packa = np.zeros((128, PACKA_N), np.float32)
    for nm in PACKA_CONST:
        a = c[nm].reshape(c[nm].shape[0], -1)
        o, r, w = PACKA_OFFS[nm]
        packa[:r, o:o + w] = a
    vals = {"g1": np.asarray(g1, np.float32).reshape(8, 1),
            "g2": np.asarray(g2, np.float32).reshape(8, 1),
            "gf": np.asarray(gf, np.float32).reshape(8, 1),
            "wq": Wq, "wk": Wk, "wv": Wv, "wo": Wo,
            "wup": Wup, "wdown": Wdown, "wvocab": Wvocab}
    for nm, (r, w) in PACKA_WEIGHT.items():
        o, _, _ = PACKA_OFFS[nm]
        packa[:r, o:o + w] = np.asarray(vals[nm], np.float32)
    packr = np.zeros((128, PACKR_N), np.float32)
    for nm in PACKR_CONST:
        a = c[nm].reshape(c[nm].shape[0], -1)
        o, r, w = PACKR_OFFS[nm]
        packr[:r, o:o + w] = a
    o, r, w = PACKR_OFFS["emb"]
    packr[:r, o:o + w] = np.asarray(emb, np.float32)
    return packa, packr


def _pack_x(x):
    """x [4, 2048] int -> paired-index stream xx [128, 32] uint16.

    Group G8 in [0,8) handles rows G8*1024 + 2i + parity, i in [0,512).
    combined index i of group G8 lives at partition 16*G8 + i%16, slot i//16.
    """
    x = np.asarray(x).astype(np.int64)
    G8 = np.arange(8)[:, None]
    i = np.arange(512)[None, :]
    b = G8 // 2
    irev = (i // 128) * 128 + (127 - i % 128)  # f-reversal within 128-chunks
    s0 = (G8 % 2) * 1024 + 2 * irev
    x0 = x[b, s0]
    x1 = x[b, s0 + 1]
    comb = (x0 * V + x1).astype(np.uint16)  # [8, 512]
    xx = np.zeros((128, 32), dtype=np.uint16)
    for g8 in range(8):
        xx[16 * g8 + (np.arange(512) % 16), np.arange(512) // 16] = comb[g8]
    return xx


# ----------------------------------------------------------------------------
# Device program
# ----------------------------------------------------------------------------

def build_nc(debug=False):
    nc = bacc.Bacc(trn_type="TRN2")
    dbg_specs = {
        "h_G": [128, 512], "h_R": [128, 512], "inv1": [128, 64],
        "kraw_sb": [128, 512], "k_R": [128, 512], "scores": [128, 128],
        "exp_t": [128, 128], "numden": [128, 40], "qfin": [1, 32],
        "hl_sb": [32, 1], "ctx": [32, 1], "h2_sb": [32, 1], "h3": [32, 1],
    }
    dbg_out = {}
    if debug:
        for nm, shp in dbg_specs.items():
            dbg_out[nm] = nc.dram_tensor("dbg_" + nm, shp, F32,
                                         kind="ExternalOutput").ap()

    def din(name, shape, dtype=F32):
        return nc.dram_tensor(name, list(shape), dtype, kind="ExternalInput").ap()

    xx = din("xx", [128, 32], U16)
    packa_d = din("packa", [128, PACKA_N], F32)
    packr_d = din("packr", [128, PACKR_N], F32)
    out_d = nc.dram_tensor("logits", [80], F32, kind="ExternalOutput").ap()

    with TileContext(nc) as tc:
        with tc.tile_pool(name="sb", bufs=1) as sb, \
             tc.tile_pool(name="psA", bufs=4, space="PSUM") as psA, \
             tc.tile_pool(name="psB", bufs=4, space="PSUM") as psB:

            xx_sb = sb.tile([128, 32], U16, tag="xx")
            nc.sync.dma_start(out=xx_sb, in_=xx)
            packa = sb.tile([128, PACKA_N], F32, tag="packa")
            nc.sync.dma_start(out=packa, in_=packa_d)
            packr = sb.tile([128, PACKR_N], F32, tag="packr")
            nc.sync.dma_start(out=packr, in_=packr_d)

            def pka(nm):
                o, r, w = PACKA_OFFS[nm]
                return packa[:r, o:o + w]

            def pkr(nm):
                o, r, w = PACKR_OFFS[nm]
                return packr[:r, o:o + w]

            mask16 = pka("mask16"); m432 = pka("m432"); m496 = pka("m496")
            m9632 = pka("m9632"); m480 = pka("m480")
            x832 = pka("x832"); x432 = pka("x432"); x480 = pka("x480")
            bdone4 = pka("bdone4"); e16 = pka("e16"); e4 = pka("e4"); e24 = pka("e24")
            g1 = pka("g1"); g2 = pka("g2"); gf = pka("gf")
            wq = pka("wq"); wk = pka("wk"); wv = pka("wv"); wo = pka("wo")
            wup = pka("wup"); wdn = pka("wdown"); wvoc = pka("wvocab")
            cos_t = pka("cos_t").rearrange("p (a b) -> p a b", b=2)
            sin_t = pka("sin_t").rearrange("p (a b) -> p a b", b=2)
            id128 = pkr("id128"); p0 = pkr("p0"); p1 = pkr("p1"); emb_sb = pkr("emb")

            ones_m = pka("ones_m"); one11 = pka("one11")
            ones_c = pka("ones_c"); eps_c = pka("eps_c")

            # ---------------- embedding product table -----------------------
            # embT [8, 20] = emb.T
            embT_ps = psB.tile([8, V], F32, tag="small")
            nc.tensor.transpose(embT_ps, emb_sb, id128[:V, :V])
            embT = sb.tile([8, V], F32, tag="embT")
            nc.scalar.copy(embT, embT_ps)
            # embT2rep [128, 400]: parity0 -> emb[v1, d], parity1 -> emb[v2, d]
            e2_ps = psA.tile([128, V * V], F32, tag="big")
            rhs_v1 = _ap(embT[:, :], [[1, V], [0, V]])   # emb[v1,d] bcast over v2
            rhs_v2 = _ap(embT[:, :], [[0, V], [1, V]])   # emb[v2,d] bcast over v1
            nc.tensor.matmul(e2_ps, p0, rhs_v1, start=True, stop=False)
            nc.tensor.matmul(e2_ps, p1, rhs_v2, start=False, stop=True)
            embT2rep = sb.tile([128, V * V], F32, tag="embT2rep")
            nc.scalar.copy(embT2rep, e2_ps)

            # ---------------- gather: h in G layout --------------------------
            h_G = sb.tile([128, 512], F32, tag="h_G")
            nc.gpsimd.indirect_copy(h_G, embT2rep, xx_sb, i_know_ap_gather_is_preferred=True)

            # ---------------- block-diagonal weights -------------------------
            # wkp = diag(g1) @ Wk ; wvp likewise ; wqp = 0.5 * diag(g1) @ Wq
            wkp = sb.tile([D, D], F32, tag="wkp")
            nc.vector.tensor_scalar_mul(wkp, wk, g1)
            wvp = sb.tile([D, D], F32, tag="wvp")
            nc.vector.tensor_scalar_mul(wvp, wv, g1)
            wqp = sb.tile([D, D], F32, tag="wqp")
            nc.vector.tensor_scalar_mul(wqp, wq, g1)
            nc.scalar.mul(wqp, wqp, 0.5)
            wupp = sb.tile([D, Fdim], F32, tag="wupp")
            nc.vector.tensor_scalar_mul(wupp, wup, g2)
            wvocp = sb.tile([D, V], F32, tag="wvocp")
            nc.vector.tensor_scalar_mul(wvocp, wvoc, gf)

            def blockdiag(w_sb, e_mat, mask, P, M, blk_p, blk_f, dtype, tag):
                """[P, M] block-diagonal: rep = E.T @ w ; bd = rep_bcast * mask."""
                rep_ps = psB.tile([P, blk_f], F32, tag="small")
                nc.tensor.matmul(rep_ps, e_mat, w_sb)
                bd = sb.tile([P, M], dtype, tag=tag)
                rep_b = _ap(rep_ps[:, :], [[0, M // blk_f], [1, blk_f]])
                nc.vector.tensor_tensor(out=bd, in0=rep_b, in1=mask, op=ALU.mult)
                return bd

            bdk = blockdiag(wkp, e16, mask16, 128, 128, 8, 8, F32, "bdk")
            bdv = blockdiag(wvp, e16, mask16, 128, 128, 8, 8, F32, "bdv")
            bdq4 = blockdiag(wqp, e4, m432, 32, 32, 8, 8, F32, "bdq4")
            bdo4 = blockdiag(wo, e4, m432, 32, 32, 8, 8, F32, "bdo4")
            bdup4 = blockdiag(wupp, e4, m496, 32, 96, 8, 24, F32, "bdup4")
            bddn4 = blockdiag(wdn, e24, m9632, 96, 32, 24, 8, F32, "bddn4")
            bdvoc4 = blockdiag(wvocp, e4, m480, 32, 80, 8, 20, F32, "bdvoc4")

            # ---------------- h -> R layout (4 transposes) -------------------
            hR_ps = psA.tile([128, 512], F32, tag="big")
            for c in range(4):
                nc.tensor.transpose(hR_ps[:, 128 * c:128 * (c + 1)],
                                    h_G[:, 128 * c:128 * (c + 1)],
                                    id128)
            h_R = sb.tile([128, 64, 8], F32, tag="h_R")
            hRf = h_R.rearrange("p a b -> p (a b)")
            nc.scalar.copy(hRf[:, 0:256], hR_ps[:, 0:256])
            nc.vector.tensor_copy(hRf[:, 256:512], hR_ps[:, 256:512])

            # ---------------- rmsnorm stats ----------------------------------
            hsq = sb.tile([128, 64, 8], F32, tag="hsq")
            nc.scalar.activation(hsq, h_R, AF.Square)
            ssq = sb.tile([128, 64], F32, tag="ssq")
            nc.vector.reduce_sum(ssq, hsq, axis=AX.X)
            rms = sb.tile([128, 64], F32, tag="rms")
            nc.scalar.activation(rms, ssq, AF.Sqrt, bias=eps_c[:, :], scale=1.0 / D)
            inv1 = sb.tile([128, 64], F32, tag="inv1")
            nc.vector.reciprocal(inv1, rms)

            # cos/sin with inv1 folded (k = rope(kraw) * inv1[row])
            cosI = sb.tile([128, 64, 2], F32, tag="cosI")
            inv1_b = _ap(inv1[:, :], [[1, 64], [0, 2]])
            nc.gpsimd.tensor_tensor(out=cosI, in0=cos_t, in1=inv1_b, op=ALU.mult)
            sinI = sb.tile([128, 64, 2], F32, tag="sinI")
            nc.gpsimd.tensor_tensor(out=sinI, in0=sin_t, in1=inv1_b, op=ALU.mult)

            # ---------------- projections (G layout) -------------------------
            kraw_ps = psA.tile([128, 512], F32, tag="big")
            nc.tensor.matmul(kraw_ps, bdk, h_G)
            vraw_ps = psA.tile([128, 512], F32, tag="big")
            nc.tensor.matmul(vraw_ps, bdv, h_G)

            kraw_sb = sb.tile([128, 512], F32, tag="kraw_sb")
            nc.scalar.copy(kraw_sb[:, 0:256], kraw_ps[:, 0:256])
            nc.vector.tensor_copy(kraw_sb[:, 256:512], kraw_ps[:, 256:512])
            vraw_sb = sb.tile([128, 512], F32, tag="vraw_sb")
            nc.scalar.copy(vraw_sb[:, 0:256], vraw_ps[:, 0:256])
            nc.vector.tensor_copy(vraw_sb[:, 256:512], vraw_ps[:, 256:512])

            kR_ps = psA.tile([128, 512], F32, tag="big")
            vR_ps = psA.tile([128, 512], F32, tag="big")
            for c in range(4):
                nc.tensor.transpose(kR_ps[:, 128 * c:128 * (c + 1)],
                                    kraw_sb[:, 128 * c:128 * (c + 1)],
                                    id128)
                nc.tensor.transpose(vR_ps[:, 128 * c:128 * (c + 1)],
                                    vraw_sb[:, 128 * c:128 * (c + 1)],
                                    id128)

            # ---------------- rope on k (R layout) ---------------------------
            # k view [128, 64 t, 2 h, 2 j, 2 par]; cosI [128, 64, 2(j)]
            kR_sb = sb.tile([128, 512], F32, tag="kR_sb")
            nc.scalar.copy(kR_sb, kR_ps)
            def kview(tile, par):
                return _ap(tile[:, :], [[8, 64], [4, 2], [2, 2], [1, 1]], extra_off=par)
            cos_b = _ap(cosI[:, :, :], [[2, 64], [0, 2], [1, 2], [0, 1]])
            sin_b = _ap(sinI[:, :, :], [[2, 64], [0, 2], [1, 2], [0, 1]])
            ke = kview(kR_sb, 0); ko = kview(kR_sb, 1)
            t1 = sb.tile([128, 64, 2, 2, 1], F32, tag="t1")
            t2 = sb.tile([128, 64, 2, 2, 1], F32, tag="t2")
            t3 = sb.tile([128, 64, 2, 2, 1], F32, tag="t3")
            t4 = sb.tile([128, 64, 2, 2, 1], F32, tag="t4")
            nc.vector.tensor_tensor(out=t1, in0=ke, in1=cos_b, op=ALU.mult)
            nc.vector.tensor_tensor(out=t2, in0=ko, in1=sin_b, op=ALU.mult)
            nc.gpsimd.tensor_tensor(out=t3, in0=ke, in1=sin_b, op=ALU.mult)
            nc.gpsimd.tensor_tensor(out=t4, in0=ko, in1=cos_b, op=ALU.mult)
            k_R = sb.tile([128, 64, 8], F32, tag="k_R")
            k_e = _ap(k_R[:, :, :], [[8, 64], [4, 2], [2, 2], [1, 1]], extra_off=0)
            k_o = _ap(k_R[:, :, :], [[8, 64], [4, 2], [2, 2], [1, 1]], extra_off=1)
            nc.vector.tensor_tensor(out=k_e, in0=t1, in1=t2, op=ALU.subtract)
            nc.gpsimd.tensor_tensor(out=k_o, in0=t3, in1=t4, op=ALU.add)

            # ---------------- q at position S-1 ------------------------------
            # h_last strip: h_R[0, 51+4b, :]  -> [1, 32] (b, d)
            hl_row = _ap(h_R[0:1, 51, :], [[32, 4], [1, 8]])
            hl_flat = sb.tile([1, 32], F32, tag="hl_flat")
            nc.vector.tensor_copy(hl_flat, hl_row)
            hl_ps = psB.tile([32, 1], F32, tag="small")
            nc.tensor.matmul(hl_ps, hl_flat, one11)
            hl_sb = sb.tile([32, 1], F32, tag="hl_sb")
            nc.scalar.copy(hl_sb, hl_ps)
            qc_ps = psB.tile([32, 1], F32, tag="small")
            nc.tensor.matmul(qc_ps, bdq4, hl_sb)
            qc_sb = sb.tile([32, 1], F32, tag="qc_sb")
            nc.scalar.copy(qc_sb, qc_ps)
            qr_ps = psB.tile([1, 32], F32, tag="small")
            nc.tensor.matmul(qr_ps, qc_sb, id128[:32, :32])
            q_row = sb.tile([1, 32], F32, tag="q_row")
            nc.scalar.copy(q_row, qr_ps)
            # scale by inv1 of the last-token rows: inv1[0, 51+4b]
            inv1_strip = _ap(inv1[0:1, 51:52], [[4, 4], [0, 8]])
            qs = sb.tile([1, 32], F32, tag="qs")
            nc.vector.tensor_tensor(out=qs, in0=_ap(q_row[:, :], [[8, 4], [1, 8]]),
                                    in1=inv1_strip, op=ALU.mult)
            # rope at s = 2047: cos/sin from tables at [0, 51, j]
            cq = _ap(cos_t[0:1, 51, :], [[0, 4], [0, 2], [1, 2]])
            sq = _ap(sin_t[0:1, 51, :], [[0, 4], [0, 2], [1, 2]])
            def qview(tile, par):
                return _ap(tile[:, :], [[8, 4], [4, 2], [2, 2]], extra_off=par)
            qe = qview(qs, 0); qo = qview(qs, 1)
            u1 = sb.tile([1, 16], F32, tag="u1"); u2 = sb.tile([1, 16], F32, tag="u2")
            u3 = sb.tile([1, 16], F32, tag="u3"); u4 = sb.tile([1, 16], F32, tag="u4")
            nc.vector.tensor_tensor(out=u1, in0=qe, in1=cq, op=ALU.mult)
            nc.vector.tensor_tensor(out=u2, in0=qo, in1=sq, op=ALU.mult)
            nc.vector.tensor_tensor(out=u3, in0=qe, in1=sq, op=ALU.mult)
            nc.vector.tensor_tensor(out=u4, in0=qo, in1=cq, op=ALU.mult)
            qfin = sb.tile([1, 32], F32, tag="qfin")
            qf_e = qview(qfin, 0); qf_o = qview(qfin, 1)
            u1v = _ap(u1[:, :], [[4, 4], [2, 2], [1, 2]])
            u2v = _ap(u2[:, :], [[4, 4], [2, 2], [1, 2]])
            u3v = _ap(u3[:, :], [[4, 4], [2, 2], [1, 2]])
            u4v = _ap(u4[:, :], [[4, 4], [2, 2], [1, 2]])
            nc.vector.tensor_tensor(out=qf_e, in0=u1v, in1=u2v, op=ALU.subtract)
            nc.vector.tensor_tensor(out=qf_o, in0=u3v, in1=u4v, op=ALU.add)

            # replicate q over all rows: qrep[p, t, d] = qfin[b(t), hhd]
            qext = sb.tile([1, 128], F32, tag="qext")
            nc.vector.tensor_copy(qext, _ap(qfin[:, :], [[8, 4], [0, 4], [1, 8]]))
            qrep_ps = psA.tile([128, 512], F32, tag="big")
            qsrc = _ap(qext[:, :], [[0, 4], [1, 128]])
            nc.tensor.matmul(qrep_ps, ones_m, qsrc)

            # ---------------- scores + softmax (no max-sub; scores bounded) --
            sprod = sb.tile([128, 64, 2, 4], F32, tag="sprod")
            nc.vector.tensor_tensor(out=sprod, in0=_ap(k_R[:, :, :], [[8, 64], [4, 2], [1, 4]]),
                                    in1=_ap(qrep_ps[:, :], [[8, 64], [4, 2], [1, 4]]),
                                    op=ALU.mult)
            scores = sb.tile([128, 64, 2], F32, tag="scores")
            nc.vector.reduce_sum(scores, sprod, axis=AX.X)
            exp_t = sb.tile([128, 64, 2], F32, tag="exp_t")
            nc.scalar.activation(exp_t, scores, AF.Exp)
            expinv = sb.tile([128, 64, 2], F32, tag="expinv")
            nc.vector.tensor_tensor(out=expinv, in0=exp_t, in1=inv1_b, op=ALU.mult)
            vw = sb.tile([128, 64, 2, 4], F32, tag="vw")
            nc.vector.tensor_tensor(out=vw, in0=_ap(vR_ps[:, :], [[8, 64], [4, 2], [1, 4]]),
                                    in1=_ap(expinv[:, :, :], [[2, 64], [1, 2], [0, 4]]),
                                    op=ALU.mult)

            # per-batch partial reductions -> numden [128, 40]
            numden = sb.tile([128, 40], F32, tag="numden")
            for b in range(B):
                nin = _ap(vw[:, :, :, :], [[1, 8], [128, 4], [8, 4]], extra_off=32 * b)
                nc.vector.reduce_sum(numden[:, 8 * b:8 * (b + 1)], nin, axis=AX.XY)
            for b in range(B):
                din_ = _ap(exp_t[:, :, :], [[1, 2], [32, 4], [2, 4]], extra_off=8 * b)
                nc.vector.reduce_sum(numden[:, 32 + 2 * b:34 + 2 * b], din_, axis=AX.XY)

            combo_ps = psB.tile([40, 1], F32, tag="small")
            nc.tensor.matmul(combo_ps, numden, ones_c)

            # ---------------- tail: ctx, attn-out, FFN, final norm ----------
            den_sb = sb.tile([8, 1], F32, tag="den_sb")
            nc.scalar.copy(den_sb, combo_ps[32:40, :])
            rden = sb.tile([8, 1], F32, tag="rden")
            nc.vector.reciprocal(rden, den_sb)
            rdx_ps = psB.tile([32, 1], F32, tag="small")
            nc.tensor.matmul(rdx_ps, x832, rden)
            num_sb = sb.tile([32, 1], F32, tag="num_sb")
            nc.scalar.copy(num_sb, combo_ps[0:32, :])
            ctx = sb.tile([32, 1], F32, tag="ctx")
            nc.vector.tensor_tensor(out=ctx, in0=num_sb, in1=rdx_ps, op=ALU.mult)

            h2_ps = psB.tile([32, 1], F32, tag="small")
            nc.tensor.matmul(h2_ps, hl_flat, one11, start=True, stop=False)
            nc.tensor.matmul(h2_ps, bdo4, ctx, start=False, stop=True)
            h2_sb = sb.tile([32, 1], F32, tag="h2_sb")
            nc.scalar.copy(h2_sb, h2_ps)

            h2sq = sb.tile([32, 1], F32, tag="h2sq")
            nc.scalar.activation(h2sq, h2_sb, AF.Square)
            ssq2_ps = psB.tile([4, 1], F32, tag="small")
            nc.tensor.matmul(ssq2_ps, bdone4, h2sq)
            rms2 = sb.tile([4, 1], F32, tag="rms2")
            nc.scalar.activation(rms2, ssq2_ps, AF.Sqrt, bias=eps_c[:4, :], scale=1.0 / D)
            inv2 = sb.tile([4, 1], F32, tag="inv2")
            nc.vector.reciprocal(inv2, rms2)

            y_ps = psB.tile([96, 1], F32, tag="small")
            nc.tensor.matmul(y_ps, bdup4, h2_sb)
            frelu = sb.tile([96, 1], F32, tag="frelu")
            nc.scalar.activation(frelu, y_ps, AF.Relu)
            dl_ps = psB.tile([32, 1], F32, tag="small")
            nc.tensor.matmul(dl_ps, bddn4, frelu)
            i2x_ps = psB.tile([32, 1], F32, tag="small")
            nc.tensor.matmul(i2x_ps, x432, inv2)
            i2x_sb = sb.tile([32, 1], F32, tag="i2x_sb")
            nc.scalar.copy(i2x_sb, i2x_ps)
            h3 = sb.tile([32, 1], F32, tag="h3")
            nc.vector.scalar_tensor_tensor(out=h3, in0=dl_ps, scalar=i2x_sb,
                                           in1=h2_sb, op0=ALU.mult, op1=ALU.add)

            h3sq = sb.tile([32, 1], F32, tag="h3sq")
            nc.scalar.activation(h3sq, h3, AF.Square)
            ssq3_ps = psB.tile([4, 1], F32, tag="small")
            nc.tensor.matmul(ssq3_ps, bdone4, h3sq)
            rms3 = sb.tile([4, 1], F32, tag="rms3")
            nc.scalar.activation(rms3, ssq3_ps, AF.Sqrt, bias=eps_c[:4, :], scale=1.0 / D)
            inv3 = sb.tile([4, 1], F32, tag="inv3")
            nc.vector.reciprocal(inv3, rms3)

            lr_ps = psB.tile([80, 1], F32, tag="small")
            nc.tensor.matmul(lr_ps, bdvoc4, h3)
            i3x_ps = psB.tile([80, 1], F32, tag="small")
            nc.tensor.matmul(i3x_ps, x480, inv3)
            i3x_sb = sb.tile([80, 1], F32, tag="i3x_sb")
            nc.scalar.copy(i3x_sb, i3x_ps)
            logits_sb = sb.tile([80, 1], F32, tag="logits_sb")
            nc.vector.tensor_tensor(out=logits_sb, in0=lr_ps, in1=i3x_sb, op=ALU.mult)
            nc.sync.dma_start(out=out_d.rearrange("(a b) -> a b", b=1), in_=logits_sb)

            if debug:
                local = locals()
                for nm in dbg_out:
                    t = local[nm]
                    flat = bass.AP(tensor=t.tensor, offset=t.offset,
                                   ap=[list(t.ap[0]), [1, t.free_size()]])
                    nc.sync.dma_start(out=dbg_out[nm], in_=flat)

    nc.finalize()
    return nc


_CACHE = {}


def _in_map(x, emb, g1, Wq, Wk, Wv, Wo, g2, Wup, Wdown, gf, Wvocab):
    packa, packr = _build_packs(emb, g1, g2, gf, Wq, Wk, Wv, Wo, Wup, Wdown, Wvocab)
    return {"xx": _pack_x(x), "packa": packa, "packr": packr}


def kernel(x, emb, g1, Wq, Wk, Wv, Wo, g2, Wup, Wdown, gf, Wvocab):
    if "nc" not in _CACHE:
        _CACHE["nc"] = build_nc()
    nc = _CACHE["nc"]
    m = _in_map(x, emb, g1, Wq, Wk, Wv, Wo, g2, Wup, Wdown, gf, Wvocab)
    in_maps = [m for _ in range(NCORES)]
    res = run_bass_kernel_spmd(nc, in_maps, core_ids=list(range(NCORES)))
    logits = np.asarray(res.results[0]["logits"], dtype=np.float32)
    return logits.reshape(B, V)


# revision 31
# speedup vs baseline: 1.3006x; 1.3006x over previous
"""Trainium2 Bass kernel for nn_MiniLLM (B=4, S=2048, D=8, H=2, HD=4, F=24, V=20).

Key insight: the reference only uses the LAST token's hidden state for the
output logits, so the S^2 attention matrix is dead code.  We need q only at
position S-1 (causal mask there admits all keys), and k/v over all B*S rows.

Layouts (per core):
  G layout: [128, 512] f32 = 16 groups x 8 dims on partitions, 512 rows free.
      partition p = g*8 + d,  g in [0,16);  free f in [0,512)
      flat row = (g//2)*1024 + 2*f + (g%2)      (b = g//4, s = row % 2048)
  R layout: [128 part, 64 t, 8 d] with t = c*16 + g (c in [0,4)), related to
      G by PE transpose of 128x128 chunks: h_R[p, c*16+g, d] = h_G[g*8+d, c*128+p]
      => s(p,c,g) = ((g%4)//2)*1024 + 256*c + 2*p + (g%2)

Embedding gather: gpsimd indirect_copy shares one index per 16-partition
group, so we gather PAIRS of rows with combined index x0*20+x1 from a
[128, 400] product table embT2rep[p, v1*20+v2] = emb[v1 or v2, p%8]
(parity = (p%16)//8 picks v1/v2) - landing h directly in G layout.
"""

import os
import sys
import numpy as np

sys.path.insert(0, "/opt/trn_rl_repo")

import concourse.bass as bass
import concourse.bacc as bacc
from concourse import mybir

jnp_bf16 = mybir.dt.np(mybir.dt.bfloat16)
from concourse.tile import TileContext
from concourse.bass_utils import run_bass_kernel_spmd

F32 = mybir.dt.float32
BF16 = mybir.dt.bfloat16
F32R = mybir.dt.float32r
U16 = mybir.dt.uint16
AF = mybir.ActivationFunctionType
ALU = mybir.AluOpType
AX = mybir.AxisListType

B, S, D, H, HD, Fdim, V = 4, 2048, 8, 2, 4, 24, 20
EPS = 1e-6
NCORES = 8


def _ap(sliced, free_dims, extra_off=0):
    """Keep the partition dim of a sliced AP, replace the free dims."""
    return bass.AP(
        tensor=sliced.tensor,
        offset=sliced.offset + extra_off,
        ap=[list(sliced.ap[0])] + [list(d) for d in free_dims],
    )


# ----------------------------------------------------------------------------
# Host-side constant tables (shape-derived only; no input data involved)
# ----------------------------------------------------------------------------

def _host_consts():
    c = {}
    # s-position per R coordinate (p, t) ; t = cc*16 + g ; f-direction reversed
    # within each 128-chunk so that s = S-1 lands on partition 0
    p = np.arange(128)[:, None]
    t = np.arange(64)[None, :]
    cc, g = t // 16, t % 16
    s = ((g % 4) // 2) * 1024 + 2 * (128 * cc + 127 - p) + (g % 2)  # [128, 64]
    freqs = 1.0 / (10000.0 ** (np.arange(0, HD, 2) / HD))  # [2] = [1.0, 0.01]
    ang = s[:, :, None].astype(np.float64) * freqs[None, None, :]
    c["cos_t"] = np.cos(ang).astype(np.float32)  # [128, 64, 2]
    c["sin_t"] = np.sin(ang).astype(np.float32)

    ii = np.arange(128)
    c["id128"] = np.eye(128, dtype=np.float32)
    # P0[d, p] = 1 iff p%16 == d ; P1: p%16 == 8+d   (embT2rep builders)
    c["p0"] = (ii[None, :] % 16 == np.arange(8)[:, None]).astype(np.float32)
    c["p1"] = (ii[None, :] % 16 == 8 + np.arange(8)[:, None]).astype(np.float32)
    # replicators E*[d, m] = 1 iff m % D == d
    c["e16"] = (np.arange(128)[None, :] % 8 == np.arange(8)[:, None]).astype(np.float32)
    c["e4"] = (np.arange(32)[None, :] % 8 == np.arange(8)[:, None]).astype(np.float32)
    c["e24"] = (np.arange(96)[None, :] % 24 == np.arange(24)[:, None]).astype(np.float32)
    # block-diag masks
    c["mask16"] = (np.arange(128)[None, :] // 8 == ii[:, None] // 8).astype(np.float32)
    c["m432"] = (np.arange(32)[None, :] // 8 == np.arange(32)[:, None] // 8).astype(np.float32)
    c["m496"] = (np.arange(96)[None, :] // 24 == np.arange(32)[:, None] // 8).astype(np.float32)
    c["m9632"] = (np.arange(32)[None, :] // 8 == np.arange(96)[:, None] // 24).astype(np.float32)
    c["m480"] = (np.arange(80)[None, :] // 20 == np.arange(32)[:, None] // 8).astype(np.float32)
    # expanders
    c["x832"] = (np.arange(32)[None, :] // 4 == np.arange(8)[:, None]).astype(np.float32)
    c["x432"] = (np.arange(32)[None, :] // 8 == np.arange(4)[:, None]).astype(np.float32)
    c["x480"] = (np.arange(80)[None, :] // 20 == np.arange(4)[:, None]).astype(np.float32)
    # bdone4[p, m] = 1 iff m == p//8   (per-batch sum over 8 dims)
    c["bdone4"] = (np.arange(4)[None, :] == np.arange(32)[:, None] // 8).astype(np.float32)
    # vcols[p, j] = 8*j + p%8  (one-hot compare values)
    c["vcols"] = (8.0 * np.arange(4)[None, :] + (np.arange(128) % 8)[:, None]).astype(np.float32)
    c["ones_m"] = np.ones((1, 128), np.float32)
    c["one11"] = np.ones((1, 1), np.float32)
    c["ones_c"] = np.ones((128, 1), np.float32)
    c["eps_c"] = np.full((128, 1), EPS, np.float32)
    return c


# packed layouts: name -> (rows, cols); weights are filled per-call
PACKA_CONST = ["mask16", "id128", "vcols", "m432", "m496", "m9632", "m480",
               "x832", "x432", "x480", "bdone4", "e16", "e4", "e24",
               "ones_m", "one11", "ones_c", "eps_c"]
PACKA_WEIGHT = {"g1": (8, 1), "g2": (8, 1), "gf": (8, 1),
                "wq": (8, 8), "wk": (8, 8), "wv": (8, 8), "wo": (8, 8),
                "wup": (8, 24), "wdown": (24, 8), "wvocab": (8, 20)}
PACKL_CONST = ["cos_t", "sin_t"]


def _pack_layout():
    c = _host_consts()
    offs, cur = {}, 0
    for nm in PACKA_CONST:
        a = c[nm].reshape(c[nm].shape[0], -1)
        offs[nm] = (cur, a.shape[0], a.shape[1])
        cur += a.shape[1]
    for nm, (r, w) in PACKA_WEIGHT.items():
        offs[nm] = (cur, r, w)
        cur += w
    na = cur
    offsl, cur = {}, 0
    for nm in PACKL_CONST:
        a = c[nm].reshape(c[nm].shape[0], -1)
        offsl[nm] = (cur, a.shape[0], a.shape[1])
        cur += a.shape[1]
    return offs, na, offsl, cur


PACKA_OFFS, PACKA_N, PACKL_OFFS, PACKL_N = _pack_layout()


def _build_packs(emb, g1, g2, gf, Wq, Wk, Wv, Wo, Wup, Wdown, Wvocab):
    c = _host_consts()
    packa = np.zeros((128, PACKA_N), np.float32)
    for nm in PACKA_CONST:
        a = c[nm].reshape(c[nm].shape[0], -1)
        o, r, w = PACKA_OFFS[nm]
        packa[:r, o:o + w] = a
    vals = {"g1": np.asarray(g1, np.float32).reshape(8, 1),
            "g2": np.asarray(g2, np.float32).reshape(8, 1),
            "gf": np.asarray(gf, np.float32).reshape(8, 1),
            "wq": Wq, "wk": Wk, "wv": Wv, "wo": Wo,
            "wup": Wup, "wdown": Wdown, "wvocab": Wvocab}
    for nm, (r, w) in PACKA_WEIGHT.items():
        o, _, _ = PACKA_OFFS[nm]
        packa[:r, o:o + w] = np.asarray(vals[nm], np.float32)
    packl = np.zeros((128, PACKL_N), np.float32)
    for nm in PACKL_CONST:
        a = c[nm].reshape(c[nm].shape[0], -1)
        o, r, w = PACKL_OFFS[nm]
        packl[:r, o:o + w] = a
    emb24 = np.zeros((24, D), np.float32)
    emb24[:V] = np.asarray(emb, np.float32)
    return packa, packl, emb24


def _pack_x(x):
    """x [4, 2048] int -> xvg [128, 512] bf16: x-value at G-layout coordinates,
    replicated down each 8-partition group: xvg[g*8+d, f] = x[row(g, f)]."""
    x = np.asarray(x).astype(np.int64)
    g = np.arange(16)[:, None]
    f = np.arange(512)[None, :]
    irev = (f // 128) * 128 + (127 - f % 128)  # f-reversal within 128-chunks
    row = (g // 2) * 1024 + 2 * irev + (g % 2)
    vals = x[row // S, row % S]  # [16, 512]
    xvg = np.repeat(vals, 8, axis=0).astype(jnp_bf16)
    return xvg


# ----------------------------------------------------------------------------
# Device program
# ----------------------------------------------------------------------------

def build_nc(debug=False):
    nc = bacc.Bacc(trn_type="TRN2")
    dbg_specs = {
        "h_G": [128, 512], "h_R": [128, 512], "inv1": [128, 64],
        "kraw_sb": [128, 512], "k_R": [128, 512], "scores": [128, 128],
        "exp_t": [128, 128], "numden": [128, 40], "qfin": [1, 32],
        "hl_sb": [32, 1], "ctx": [32, 1], "h2_sb": [32, 1], "h3": [32, 1],
    }
    dbg_out = {}
    if debug:
        for nm, shp in dbg_specs.items():
            dbg_out[nm] = nc.dram_tensor("dbg_" + nm, shp, F32,
                                         kind="ExternalOutput").ap()

    def din(name, shape, dtype=F32):
        return nc.dram_tensor(name, list(shape), dtype, kind="ExternalInput").ap()

    xvg_d = din("xvg", [128, 512], BF16)
    packa_d = din("packa", [128, PACKA_N], F32)
    packl_d = din("packl", [128, PACKL_N], F32)
    emb_d = din("emb24", [24, D], F32)
    out_d = nc.dram_tensor("logits", [80], F32, kind="ExternalOutput").ap()

    with TileContext(nc) as tc:
        with tc.tile_pool(name="sb", bufs=1) as sb, \
             tc.tile_pool(name="psA", bufs=3, space="PSUM") as psA, \
             tc.tile_pool(name="psB", bufs=4, space="PSUM") as psB, \
             tc.tile_pool(name="psH", bufs=1, space="PSUM") as psH:

            packa = sb.tile([128, PACKA_N], F32, tag="packa")
            nc.sync.dma_start(out=packa, in_=packa_d)
            xvg = sb.tile([128, 512], BF16, tag="xvg")
            nc.sync.dma_start(out=xvg, in_=xvg_d)
            packl = sb.tile([128, PACKL_N], F32, tag="packl")
            nc.sync.dma_start(out=packl, in_=packl_d)
            embj = []
            for j in range(3):
                t = sb.tile([8, D], F32, tag=f"embj{j}")
                nc.sync.dma_start(out=t, in_=emb_d[8 * j:8 * (j + 1), :])
                embj.append(t)

            def pka(nm):
                o, r, w = PACKA_OFFS[nm]
                return packa[:r, o:o + w]

            mask16 = pka("mask16"); m432 = pka("m432"); m496 = pka("m496")
            m9632 = pka("m9632"); m480 = pka("m480")
            x832 = pka("x832"); x432 = pka("x432"); x480 = pka("x480")
            bdone4 = pka("bdone4"); e16 = pka("e16"); e4 = pka("e4"); e24 = pka("e24")
            g1 = pka("g1"); g2 = pka("g2"); gf = pka("gf")
            wq = pka("wq"); wk = pka("wk"); wv = pka("wv"); wo = pka("wo")
            wup = pka("wup"); wdn = pka("wdown"); wvoc = pka("wvocab")
            vcols = pka("vcols")
            _lo = PACKL_OFFS["cos_t"][0]
            cos_t = packl[:, _lo:_lo + 128].rearrange("p (a b) -> p a b", b=2)
            _lo = PACKL_OFFS["sin_t"][0]
            sin_t = packl[:, _lo:_lo + 128].rearrange("p (a b) -> p a b", b=2)
            id128 = pka("id128")
            ones_m = pka("ones_m"); one11 = pka("one11")
            ones_c = pka("ones_c"); eps_c = pka("eps_c")

            # ---------------- ACT table prewarm ------------------------------
            warm = sb.tile([1, 4], F32, tag="warm")
            nc.scalar.activation(warm[:, 0:1], one11, AF.Exp, bias=eps_c[:1, :])
            nc.scalar.activation(warm[:, 1:2], one11, AF.Sqrt, bias=eps_c[:1, :])
            nc.scalar.activation(warm[:, 2:3], one11, AF.Square, bias=eps_c[:1, :])

            # ---------------- embedding via one-hot matmuls ------------------
            # u_j[(g,vlo), f] = (x[row] == 8j+vlo);  h_G = sum_j EB_j.T @ u_j
            u_ts = []
            for j in range(3):
                u = sb.tile([128, 512], F32, tag=f"u{j}")
                nc.vector.tensor_scalar(out=u, in0=xvg, scalar1=vcols[:, j:j + 1],
                                        scalar2=None, op0=ALU.is_equal)
                u_ts.append(u)

            # ---------------- block-diagonal weights -------------------------
            # wkp = diag(g1) @ Wk ; wvp likewise ; wqp = 0.5 * diag(g1) @ Wq
            wkp = sb.tile([D, D], F32, tag="wkp")
            nc.vector.tensor_scalar_mul(wkp, wk, g1)
            wvp = sb.tile([D, D], F32, tag="wvp")
            nc.vector.tensor_scalar_mul(wvp, wv, g1)
            wqp = sb.tile([D, D], F32, tag="wqp")
            nc.vector.tensor_scalar_mul(wqp, wq, g1)
            nc.scalar.mul(wqp, wqp, 0.5)
            wupp = sb.tile([D, Fdim], F32, tag="wupp")
            nc.vector.tensor_scalar_mul(wupp, wup, g2)
            wvocp = sb.tile([D, V], F32, tag="wvocp")
            nc.vector.tensor_scalar_mul(wvocp, wvoc, gf)

            def blockdiag(w_sb, e_mat, mask, P, M, blk_p, blk_f, dtype, tag):
                """[P, M] block-diagonal: rep = E.T @ w ; bd = rep_bcast * mask."""
                rep_ps = psB.tile([P, blk_f], F32, tag="small")
                nc.tensor.matmul(rep_ps, e_mat, w_sb)
                bd = sb.tile([P, M], dtype, tag=tag)
                rep_b = _ap(rep_ps[:, :], [[0, M // blk_f], [1, blk_f]])
                nc.vector.tensor_tensor(out=bd, in0=rep_b, in1=mask, op=ALU.mult)
                return bd

            bdk = blockdiag(wkp, e16, mask16, 128, 128, 8, 8, F32, "bdk")
            bdv = blockdiag(wvp, e16, mask16, 128, 128, 8, 8, F32, "bdv")
            bdq4 = blockdiag(wqp, e4, m432, 32, 32, 8, 8, F32, "bdq4")
            bdo4 = blockdiag(wo, e4, m432, 32, 32, 8, 8, F32, "bdo4")
            bdup4 = blockdiag(wupp, e4, m496, 32, 96, 8, 24, F32, "bdup4")
            bddn4 = blockdiag(wdn, e24, m9632, 96, 32, 24, 8, F32, "bddn4")
            bdvoc4 = blockdiag(wvocp, e4, m480, 32, 80, 8, 20, F32, "bdvoc4")
            ebj = [blockdiag(embj[j], e16, mask16, 128, 128, 8, 8, F32, f"eb{j}")
                   for j in range(3)]

            hG_ps = psH.tile([128, 512], F32, tag="hg")
            for j in range(3):
                nc.tensor.matmul(hG_ps, ebj[j], u_ts[j], start=(j == 0), stop=(j == 2))
            h_G = sb.tile([128, 512], F32, tag="h_G")
            nc.scalar.copy(h_G[:, 0:256], hG_ps[:, 0:256])
            nc.vector.tensor_copy(h_G[:, 256:512], hG_ps[:, 256:512])

            # ---------------- h -> R layout (4 transposes) -------------------
            hR_ps = psA.tile([128, 512], F32, tag="big")
            for c in range(4):
                nc.tensor.transpose(hR_ps[:, 128 * c:128 * (c + 1)],
                                    h_G[:, 128 * c:128 * (c + 1)],
                                    id128)
            h_R = sb.tile([128, 64, 8], F32, tag="h_R")
            hRf = h_R.rearrange("p a b -> p (a b)")
            nc.scalar.copy(hRf[:, 0:256], hR_ps[:, 0:256])
            nc.vector.tensor_copy(hRf[:, 256:512], hR_ps[:, 256:512])

            # ---------------- rmsnorm stats ----------------------------------
            hsq = sb.tile([128, 64, 8], F32, tag="hsq")
            nc.scalar.activation(hsq, h_R, AF.Square)
            ssq = sb.tile([128, 64], F32, tag="ssq")
            nc.vector.reduce_sum(ssq, hsq, axis=AX.X)
            rms = sb.tile([128, 64], F32, tag="rms")
            nc.scalar.activation(rms, ssq, AF.Sqrt, bias=eps_c[:, :], scale=1.0 / D)
            inv1 = sb.tile([128, 64], F32, tag="inv1")
            nc.vector.reciprocal(inv1, rms)

            # cos/sin with inv1 folded (k = rope(kraw) * inv1[row])
            cosI = sb.tile([128, 64, 2], F32, tag="cosI")
            inv1_b = _ap(inv1[:, :], [[1, 64], [0, 2]])
            nc.gpsimd.tensor_tensor(out=cosI, in0=cos_t, in1=inv1_b, op=ALU.mult)
            sinI = sb.tile([128, 64, 2], F32, tag="sinI")
            nc.gpsimd.tensor_tensor(out=sinI, in0=sin_t, in1=inv1_b, op=ALU.mult)

            # ---------------- projections (G layout) -------------------------
            kraw_ps = psA.tile([128, 512], F32, tag="big")
            nc.tensor.matmul(kraw_ps, bdk, h_G)
            vraw_ps = psA.tile([128, 512], F32, tag="big")
            nc.tensor.matmul(vraw_ps, bdv, h_G)

            kraw_sb = sb.tile([128, 512], F32, tag="kraw_sb")
            nc.scalar.copy(kraw_sb[:, 0:256], kraw_ps[:, 0:256])
            nc.vector.tensor_copy(kraw_sb[:, 256:512], kraw_ps[:, 256:512])
            vraw_sb = sb.tile([128, 512], F32, tag="vraw_sb")
            nc.scalar.copy(vraw_sb[:, 0:256], vraw_ps[:, 0:256])
            nc.vector.tensor_copy(vraw_sb[:, 256:512], vraw_ps[:, 256:512])

            kR_ps = psA.tile([128, 512], F32, tag="big")
            vR_ps = psA.tile([128, 512], F32, tag="big")
            for c in range(4):
                nc.tensor.transpose(kR_ps[:, 128 * c:128 * (c + 1)],
                                    kraw_sb[:, 128 * c:128 * (c + 1)],
                                    id128)
                nc.tensor.transpose(vR_ps[:, 128 * c:128 * (c + 1)],
                                    vraw_sb[:, 128 * c:128 * (c + 1)],
                                    id128)

            # ---------------- rope on k (R layout) ---------------------------
            # k view [128, 64 t, 2 h, 2 j, 2 par]; cosI [128, 64, 2(j)]
            kR_sb = sb.tile([128, 512], F32, tag="kR_sb")
            nc.scalar.copy(kR_sb, kR_ps)
            def kview(tile, par):
                return _ap(tile[:, :], [[8, 64], [4, 2], [2, 2], [1, 1]], extra_off=par)
            cos_b = _ap(cosI[:, :, :], [[2, 64], [0, 2], [1, 2], [0, 1]])
            sin_b = _ap(sinI[:, :, :], [[2, 64], [0, 2], [1, 2], [0, 1]])
            ke = kview(kR_sb, 0); ko = kview(kR_sb, 1)
            t1 = sb.tile([128, 64, 2, 2, 1], F32, tag="t1")
            t2 = sb.tile([128, 64, 2, 2, 1], F32, tag="t2")
            t3 = sb.tile([128, 64, 2, 2, 1], F32, tag="t3")
            t4 = sb.tile([128, 64, 2, 2, 1], F32, tag="t4")
            nc.vector.tensor_tensor(out=t1, in0=ke, in1=cos_b, op=ALU.mult)
            nc.vector.tensor_tensor(out=t2, in0=ko, in1=sin_b, op=ALU.mult)
            nc.gpsimd.tensor_tensor(out=t3, in0=ke, in1=sin_b, op=ALU.mult)
            nc.gpsimd.tensor_tensor(out=t4, in0=ko, in1=cos_b, op=ALU.mult)
            k_R = sb.tile([128, 64, 8], F32, tag="k_R")
            k_e = _ap(k_R[:, :, :], [[8, 64], [4, 2], [2, 2], [1, 1]], extra_off=0)
            k_o = _ap(k_R[:, :, :], [[8, 64], [4, 2], [2, 2], [1, 1]], extra_off=1)
            nc.vector.tensor_tensor(out=k_e, in0=t1, in1=t2, op=ALU.subtract)
            nc.gpsimd.tensor_tensor(out=k_o, in0=t3, in1=t4, op=ALU.add)

            # ---------------- q at position S-1 ------------------------------
            # h_last strip: h_R[0, 51+4b, :]  -> [1, 32] (b, d)
            hl_row = _ap(h_R[0:1, 51, :], [[32, 4], [1, 8]])
            hl_flat = sb.tile([1, 32], F32, tag="hl_flat")
            nc.vector.tensor_copy(hl_flat, hl_row)
            hl_ps = psB.tile([32, 1], F32, tag="small")
            nc.tensor.matmul(hl_ps, hl_flat, one11)
            hl_sb = sb.tile([32, 1], F32, tag="hl_sb")
            nc.scalar.copy(hl_sb, hl_ps)
            qc_ps = psB.tile([32, 1], F32, tag="small")
            nc.tensor.matmul(qc_ps, bdq4, hl_sb)
            qc_sb = sb.tile([32, 1], F32, tag="qc_sb")
            nc.scalar.copy(qc_sb, qc_ps)
            qr_ps = psB.tile([1, 32], F32, tag="small")
            nc.tensor.matmul(qr_ps, qc_sb, id128[:32, :32])
            q_row = sb.tile([1, 32], F32, tag="q_row")
            nc.scalar.copy(q_row, qr_ps)
            # scale by inv1 of the last-token rows: inv1[0, 51+4b]
            inv1_strip = _ap(inv1[0:1, 51:52], [[4, 4], [0, 8]])
            qs = sb.tile([1, 32], F32, tag="qs")
            nc.vector.tensor_tensor(out=qs, in0=_ap(q_row[:, :], [[8, 4], [1, 8]]),
                                    in1=inv1_strip, op=ALU.mult)
            # rope at s = 2047: cos/sin from tables at [0, 51, j]
            cq = _ap(cos_t[0:1, 51, :], [[0, 4], [0, 2], [1, 2]])
            sq = _ap(sin_t[0:1, 51, :], [[0, 4], [0, 2], [1, 2]])
            def qview(tile, par):
                return _ap(tile[:, :], [[8, 4], [4, 2], [2, 2]], extra_off=par)
            qe = qview(qs, 0); qo = qview(qs, 1)
            u1 = sb.tile([1, 16], F32, tag="u1"); u2 = sb.tile([1, 16], F32, tag="u2")
            u3 = sb.tile([1, 16], F32, tag="u3"); u4 = sb.tile([1, 16], F32, tag="u4")
            nc.vector.tensor_tensor(out=u1, in0=qe, in1=cq, op=ALU.mult)
            nc.vector.tensor_tensor(out=u2, in0=qo, in1=sq, op=ALU.mult)
            nc.vector.tensor_tensor(out=u3, in0=qe, in1=sq, op=ALU.mult)
            nc.vector.tensor_tensor(out=u4, in0=qo, in1=cq, op=ALU.mult)
            qfin = sb.tile([1, 32], F32, tag="qfin")
            qf_e = qview(qfin, 0); qf_o = qview(qfin, 1)
            u1v = _ap(u1[:, :], [[4, 4], [2, 2], [1, 2]])
            u2v = _ap(u2[:, :], [[4, 4], [2, 2], [1, 2]])
            u3v = _ap(u3[:, :], [[4, 4], [2, 2], [1, 2]])
            u4v = _ap(u4[:, :], [[4, 4], [2, 2], [1, 2]])
            nc.vector.tensor_tensor(out=qf_e, in0=u1v, in1=u2v, op=ALU.subtract)
            nc.vector.tensor_tensor(out=qf_o, in0=u3v, in1=u4v, op=ALU.add)

            # replicate q over all rows: qrep[p, t, d] = qfin[b(t), hhd]
            qext = sb.tile([1, 128], F32, tag="qext")
            nc.vector.tensor_copy(qext, _ap(qfin[:, :], [[8, 4], [0, 4], [1, 8]]))
            qrep_ps = psA.tile([128, 512], F32, tag="big")
            qsrc = _ap(qext[:, :], [[0, 4], [1, 128]])
            nc.tensor.matmul(qrep_ps, ones_m, qsrc)

            # ---------------- scores + softmax (no max-sub; scores bounded) --
            sprod = sb.tile([128, 64, 2, 4], F32, tag="sprod")
            nc.vector.tensor_tensor(out=sprod, in0=_ap(k_R[:, :, :], [[8, 64], [4, 2], [1, 4]]),
                                    in1=_ap(qrep_ps[:, :], [[8, 64], [4, 2], [1, 4]]),
                                    op=ALU.mult)
            scores = sb.tile([128, 64, 2], F32, tag="scores")
            nc.vector.reduce_sum(scores, sprod, axis=AX.X)
            exp_t = sb.tile([128, 64, 2], F32, tag="exp_t")
            nc.scalar.activation(exp_t, scores, AF.Exp)
            expinv = sb.tile([128, 64, 2], F32, tag="expinv")
            nc.vector.tensor_tensor(out=expinv, in0=exp_t, in1=inv1_b, op=ALU.mult)
            vw = sb.tile([128, 64, 2, 4], F32, tag="vw")
            nc.vector.tensor_tensor(out=vw, in0=_ap(vR_ps[:, :], [[8, 64], [4, 2], [1, 4]]),
                                    in1=_ap(expinv[:, :, :], [[2, 64], [1, 2], [0, 4]]),
                                    op=ALU.mult)

            # per-batch partial reductions -> numden [128, 40]
            numden = sb.tile([128, 40], F32, tag="numden")
            for b in range(B):
                nin = _ap(vw[:, :, :, :], [[1, 8], [128, 4], [8, 4]], extra_off=32 * b)
                nc.vector.reduce_sum(numden[:, 8 * b:8 * (b + 1)], nin, axis=AX.XY)
            for b in range(B):
                din_ = _ap(exp_t[:, :, :], [[1, 2], [32, 4], [2, 4]], extra_off=8 * b)
                nc.vector.reduce_sum(numden[:, 32 + 2 * b:34 + 2 * b], din_, axis=AX.XY)

            combo_ps = psB.tile([40, 1], F32, tag="small")
            nc.tensor.matmul(combo_ps, numden, ones_c)

            # ---------------- tail: ctx, attn-out, FFN, final norm ----------
            den_sb = sb.tile([8, 1], F32, tag="den_sb")
            nc.scalar.copy(den_sb, combo_ps[32:40, :])
            rden = sb.tile([8, 1], F32, tag="rden")
            nc.vector.reciprocal(rden, den_sb)
            rdx_ps = psB.tile([32, 1], F32, tag="small")
            nc.tensor.matmul(rdx_ps, x832, rden)
            num_sb = sb.tile([32, 1], F32, tag="num_sb")
            nc.scalar.copy(num_sb, combo_ps[0:32, :])
            ctx = sb.tile([32, 1], F32, tag="ctx")
            nc.vector.tensor_tensor(out=ctx, in0=num_sb, in1=rdx_ps, op=ALU.mult)

            h2_ps = psB.tile([32, 1], F32, tag="small")
            nc.tensor.matmul(h2_ps, hl_flat, one11, start=True, stop=False)
            nc.tensor.matmul(h2_ps, bdo4, ctx, start=False, stop=True)
            h2_sb = sb.tile([32, 1], F32, tag="h2_sb")
            nc.scalar.copy(h2_sb, h2_ps)

            h2sq = sb.tile([32, 1], F32, tag="h2sq")
            nc.vector.tensor_mul(h2sq, h2_sb, h2_sb)
            ssq2_ps = psB.tile([4, 1], F32, tag="small")
            nc.tensor.matmul(ssq2_ps, bdone4, h2sq)
            rms2 = sb.tile([4, 1], F32, tag="rms2")
            nc.scalar.activation(rms2, ssq2_ps, AF.Sqrt, bias=eps_c[:4, :], scale=1.0 / D)
            inv2 = sb.tile([4, 1], F32, tag="inv2")
            nc.vector.reciprocal(inv2, rms2)

            y_ps = psB.tile([96, 1], F32, tag="small")
            nc.tensor.matmul(y_ps, bdup4, h2_sb)
            frelu = sb.tile([96, 1], F32, tag="frelu")
            nc.vector.tensor_scalar(out=frelu, in0=y_ps, scalar1=0.0, scalar2=None, op0=ALU.max)
            dl_ps = psB.tile([32, 1], F32, tag="small")
            nc.tensor.matmul(dl_ps, bddn4, frelu)
            i2x_ps = psB.tile([32, 1], F32, tag="small")
            nc.tensor.matmul(i2x_ps, x432, inv2)
            i2x_sb = sb.tile([32, 1], F32, tag="i2x_sb")
            nc.scalar.copy(i2x_sb, i2x_ps)
            h3 = sb.tile([32, 1], F32, tag="h3")
            nc.vector.scalar_tensor_tensor(out=h3, in0=dl_ps, scalar=i2x_sb,
                                           in1=h2_sb, op0=ALU.mult, op1=ALU.add)

            h3sq = sb.tile([32, 1], F32, tag="h3sq")
            nc.vector.tensor_mul(h3sq, h3, h3)
            ssq3_ps = psB.tile([4, 1], F32, tag="small")
            nc.tensor.matmul(ssq3_ps, bdone4, h3sq)
            rms3 = sb.tile([4, 1], F32, tag="rms3")
            nc.scalar.activation(rms3, ssq3_ps, AF.Sqrt, bias=eps_c[:4, :], scale=1.0 / D)
            inv3 = sb.tile([4, 1], F32, tag="inv3")
            nc.vector.reciprocal(inv3, rms3)

            lr_ps = psB.tile([80, 1], F32, tag="small")
            nc.tensor.matmul(lr_ps, bdvoc4, h3)
            i3x_ps = psB.tile([80, 1], F32, tag="small")
            nc.tensor.matmul(i3x_ps, x480, inv3)
            i3x_sb = sb.tile([80, 1], F32, tag="i3x_sb")
            nc.scalar.copy(i3x_sb, i3x_ps)
            logits_sb = sb.tile([80, 1], F32, tag="logits_sb")
            nc.vector.tensor_tensor(out=logits_sb, in0=lr_ps, in1=i3x_sb, op=ALU.mult)
            nc.sync.dma_start(out=out_d.rearrange("(a b) -> a b", b=1), in_=logits_sb)

            if debug:
                local = locals()
                for nm in dbg_out:
                    t = local[nm]
                    flat = bass.AP(tensor=t.tensor, offset=t.offset,
                                   ap=[list(t.ap[0]), [1, t.free_size()]])
                    nc.sync.dma_start(out=dbg_out[nm], in_=flat)

    nc.finalize()
    return nc


_CACHE = {}


def _in_map(x, emb, g1, Wq, Wk, Wv, Wo, g2, Wup, Wdown, gf, Wvocab):
    packa, packl, emb24 = _build_packs(emb, g1, g2, gf, Wq, Wk, Wv, Wo, Wup, Wdown, Wvocab)
    return {"xvg": _pack_x(x), "packa": packa, "packl": packl, "emb24": emb24}


def kernel(x, emb, g1, Wq, Wk, Wv, Wo, g2, Wup, Wdown, gf, Wvocab):
    if "nc" not in _CACHE:
        _CACHE["nc"] = build_nc()
    nc = _CACHE["nc"]
    m = _in_map(x, emb, g1, Wq, Wk, Wv, Wo, g2, Wup, Wdown, gf, Wvocab)
    in_maps = [m for _ in range(NCORES)]
    res = run_bass_kernel_spmd(nc, in_maps, core_ids=list(range(NCORES)))
    logits = np.asarray(res.results[0]["logits"], dtype=np.float32)
    return logits.reshape(B, V)


# revision 32
# speedup vs baseline: 1.3927x; 1.0709x over previous
"""Trainium2 Bass kernel for nn_MiniLLM (B=4, S=2048, D=8, H=2, HD=4, F=24, V=20).

Key insight: the reference only uses the LAST token's hidden state for the
output logits, so the S^2 attention matrix is dead code.  We need q only at
position S-1 (causal mask there admits all keys), and k/v over all B*S rows.

Layouts (per core):
  G layout: [128, 512] f32 = 16 groups x 8 dims on partitions, 512 rows free.
      partition p = g*8 + d,  g in [0,16);  free f in [0,512)
      flat row = (g//2)*1024 + 2*f + (g%2)      (b = g//4, s = row % 2048)
  R layout: [128 part, 64 t, 8 d] with t = c*16 + g (c in [0,4)), related to
      G by PE transpose of 128x128 chunks: h_R[p, c*16+g, d] = h_G[g*8+d, c*128+p]
      => s(p,c,g) = ((g%4)//2)*1024 + 256*c + 2*p + (g%2)

Embedding gather: gpsimd indirect_copy shares one index per 16-partition
group, so we gather PAIRS of rows with combined index x0*20+x1 from a
[128, 400] product table embT2rep[p, v1*20+v2] = emb[v1 or v2, p%8]
(parity = (p%16)//8 picks v1/v2) - landing h directly in G layout.
"""

import os
import sys
import numpy as np

sys.path.insert(0, "/opt/trn_rl_repo")

import concourse.bass as bass
import concourse.bacc as bacc
from concourse import mybir

jnp_bf16 = mybir.dt.np(mybir.dt.bfloat16)
from concourse.tile import TileContext
from concourse.bass_utils import run_bass_kernel_spmd

F32 = mybir.dt.float32
BF16 = mybir.dt.bfloat16
F32R = mybir.dt.float32r
U16 = mybir.dt.uint16
AF = mybir.ActivationFunctionType
ALU = mybir.AluOpType
AX = mybir.AxisListType

B, S, D, H, HD, Fdim, V = 4, 2048, 8, 2, 4, 24, 20
EPS = 1e-6
NCORES = 8


def _ap(sliced, free_dims, extra_off=0):
    """Keep the partition dim of a sliced AP, replace the free dims."""
    return bass.AP(
        tensor=sliced.tensor,
        offset=sliced.offset + extra_off,
        ap=[list(sliced.ap[0])] + [list(d) for d in free_dims],
    )


# ----------------------------------------------------------------------------
# Host-side constant tables (shape-derived only; no input data involved)
# ----------------------------------------------------------------------------

def _host_consts():
    c = {}
    # s-position per R coordinate (p, t) ; t = cc*16 + g ; f-direction reversed
    # within each 128-chunk so that s = S-1 lands on partition 0
    p = np.arange(128)[:, None]
    t = np.arange(64)[None, :]
    cc, g = t // 16, t % 16
    s = ((g % 4) // 2) * 1024 + 2 * (128 * cc + 127 - p) + (g % 2)  # [128, 64]
    freqs = 1.0 / (10000.0 ** (np.arange(0, HD, 2) / HD))  # [2] = [1.0, 0.01]
    ang = s[:, :, None].astype(np.float64) * freqs[None, None, :]
    c["cos_t"] = np.cos(ang).astype(np.float32)  # [128, 64, 2]
    c["sin_t"] = np.sin(ang).astype(np.float32)

    ii = np.arange(128)
    c["id128"] = np.eye(128, dtype=np.float32)
    # P0[d, p] = 1 iff p%16 == d ; P1: p%16 == 8+d   (embT2rep builders)
    c["p0"] = (ii[None, :] % 16 == np.arange(8)[:, None]).astype(np.float32)
    c["p1"] = (ii[None, :] % 16 == 8 + np.arange(8)[:, None]).astype(np.float32)
    # replicators E*[d, m] = 1 iff m % D == d
    c["e16"] = (np.arange(128)[None, :] % 8 == np.arange(8)[:, None]).astype(np.float32)
    c["e4"] = (np.arange(32)[None, :] % 8 == np.arange(8)[:, None]).astype(np.float32)
    c["e24"] = (np.arange(96)[None, :] % 24 == np.arange(24)[:, None]).astype(np.float32)
    # block-diag masks
    c["mask16"] = (np.arange(128)[None, :] // 8 == ii[:, None] // 8).astype(np.float32)
    c["m432"] = (np.arange(32)[None, :] // 8 == np.arange(32)[:, None] // 8).astype(np.float32)
    c["m496"] = (np.arange(96)[None, :] // 24 == np.arange(32)[:, None] // 8).astype(np.float32)
    c["m9632"] = (np.arange(32)[None, :] // 8 == np.arange(96)[:, None] // 24).astype(np.float32)
    c["m480"] = (np.arange(80)[None, :] // 20 == np.arange(32)[:, None] // 8).astype(np.float32)
    # expanders
    c["x832"] = (np.arange(32)[None, :] // 4 == np.arange(8)[:, None]).astype(np.float32)
    c["x432"] = (np.arange(32)[None, :] // 8 == np.arange(4)[:, None]).astype(np.float32)
    c["x480"] = (np.arange(80)[None, :] // 20 == np.arange(4)[:, None]).astype(np.float32)
    # bdone4[p, m] = 1 iff m == p//8   (per-batch sum over 8 dims)
    c["bdone4"] = (np.arange(4)[None, :] == np.arange(32)[:, None] // 8).astype(np.float32)
    # e16j[r, j*128+m] = 1 iff r == 8j + m%8  (emb-slice replicators)
    r_ = np.arange(24)[:, None]
    jm = np.arange(3 * 128)[None, :]
    c["e16j"] = (r_ == 8 * (jm // 128) + jm % 8).astype(np.float32)
    # vcols[p, j] = 8*j + p%8  (one-hot compare values)
    c["vcols"] = (8.0 * np.arange(4)[None, :] + (np.arange(128) % 8)[:, None]).astype(np.float32)
    c["ones_m"] = np.ones((1, 128), np.float32)
    c["one11"] = np.ones((1, 1), np.float32)
    c["ones_c"] = np.ones((128, 1), np.float32)
    c["eps_c"] = np.full((128, 1), EPS, np.float32)
    return c


# packed layouts: name -> (rows, cols); weights are filled per-call
PACKA_CONST = ["mask16", "id128", "vcols", "m432", "m496", "m9632", "m480",
               "x832", "x432", "x480", "bdone4", "e16", "e4", "e24", "e16j",
               "ones_m", "one11", "ones_c", "eps_c"]
PACKA_WEIGHT = {"g1": (8, 1), "g2": (8, 1), "gf": (8, 1),
                "wq": (8, 8), "wk": (8, 8), "wv": (8, 8), "wo": (8, 8),
                "wup": (8, 24), "wdown": (24, 8), "wvocab": (8, 20)}
PACKL_CONST = ["cos_t", "sin_t"]


def _pack_layout():
    c = _host_consts()
    offs, cur = {}, 0
    for nm in PACKA_CONST:
        a = c[nm].reshape(c[nm].shape[0], -1)
        offs[nm] = (cur, a.shape[0], a.shape[1])
        cur += a.shape[1]
    for nm, (r, w) in PACKA_WEIGHT.items():
        offs[nm] = (cur, r, w)
        cur += w
    na = cur
    offsl, cur = {}, 0
    for nm in PACKL_CONST:
        a = c[nm].reshape(c[nm].shape[0], -1)
        offsl[nm] = (cur, a.shape[0], a.shape[1])
        cur += a.shape[1]
    return offs, na, offsl, cur


PACKA_OFFS, PACKA_N, PACKL_OFFS, PACKL_N = _pack_layout()


def _build_packs(emb, g1, g2, gf, Wq, Wk, Wv, Wo, Wup, Wdown, Wvocab):
    c = _host_consts()
    packa = np.zeros((128, PACKA_N), np.float32)
    for nm in PACKA_CONST:
        a = c[nm].reshape(c[nm].shape[0], -1)
        o, r, w = PACKA_OFFS[nm]
        packa[:r, o:o + w] = a
    vals = {"g1": np.asarray(g1, np.float32).reshape(8, 1),
            "g2": np.asarray(g2, np.float32).reshape(8, 1),
            "gf": np.asarray(gf, np.float32).reshape(8, 1),
            "wq": Wq, "wk": Wk, "wv": Wv, "wo": Wo,
            "wup": Wup, "wdown": Wdown, "wvocab": Wvocab}
    for nm, (r, w) in PACKA_WEIGHT.items():
        o, _, _ = PACKA_OFFS[nm]
        packa[:r, o:o + w] = np.asarray(vals[nm], np.float32)
    packl = np.zeros((128, PACKL_N), np.float32)
    for nm in PACKL_CONST:
        a = c[nm].reshape(c[nm].shape[0], -1)
        o, r, w = PACKL_OFFS[nm]
        packl[:r, o:o + w] = a
    emb24 = np.zeros((24, D), np.float32)
    emb24[:V] = np.asarray(emb, np.float32)
    return packa, packl, emb24


def _pack_x(x):
    """x [4, 2048] int -> xvg [128, 512] bf16: x-value at G-layout coordinates,
    replicated down each 8-partition group: xvg[g*8+d, f] = x[row(g, f)]."""
    x = np.asarray(x).astype(np.int64)
    g = np.arange(16)[:, None]
    f = np.arange(512)[None, :]
    irev = (f // 128) * 128 + (127 - f % 128)  # f-reversal within 128-chunks
    row = (g // 2) * 1024 + 2 * irev + (g % 2)
    vals = x[row // S, row % S]  # [16, 512]
    xvg = np.repeat(vals, 8, axis=0).astype(jnp_bf16)
    return xvg


# ----------------------------------------------------------------------------
# Device program
# ----------------------------------------------------------------------------

def build_nc(debug=False):
    nc = bacc.Bacc(trn_type="TRN2")
    dbg_specs = {
        "h_G": [128, 512], "h_R": [128, 512], "inv1": [128, 64],
        "kraw_sb": [128, 512], "k_R": [128, 512], "scores": [128, 128],
        "exp_t": [128, 128], "numden": [128, 40], "qfin": [1, 32],
        "hl_sb": [32, 1], "ctx": [32, 1], "h2_sb": [32, 1], "h3": [32, 1],
    }
    dbg_out = {}
    if debug:
        for nm, shp in dbg_specs.items():
            dbg_out[nm] = nc.dram_tensor("dbg_" + nm, shp, F32,
                                         kind="ExternalOutput").ap()

    def din(name, shape, dtype=F32):
        return nc.dram_tensor(name, list(shape), dtype, kind="ExternalInput").ap()

    xvg_d = din("xvg", [128, 512], BF16)
    packa_d = din("packa", [128, PACKA_N], F32)
    packl_d = din("packl", [128, PACKL_N], F32)
    emb_d = din("emb24", [24, D], F32)
    out_d = nc.dram_tensor("logits", [80], F32, kind="ExternalOutput").ap()

    with TileContext(nc) as tc:
        with tc.tile_pool(name="sb", bufs=1) as sb, \
             tc.tile_pool(name="psA", bufs=3, space="PSUM") as psA, \
             tc.tile_pool(name="psB", bufs=4, space="PSUM") as psB, \
             tc.tile_pool(name="psH", bufs=1, space="PSUM") as psH:

            packa = sb.tile([128, PACKA_N], F32, tag="packa")
            nc.sync.dma_start(out=packa, in_=packa_d)
            xvg = sb.tile([128, 512], BF16, tag="xvg")
            nc.gpsimd.dma_start(out=xvg, in_=xvg_d)
            packl = sb.tile([128, PACKL_N], F32, tag="packl")
            nc.gpsimd.dma_start(out=packl, in_=packl_d)
            emb24 = sb.tile([24, D], F32, tag="emb24")
            nc.sync.dma_start(out=emb24, in_=emb_d)

            def pka(nm):
                o, r, w = PACKA_OFFS[nm]
                return packa[:r, o:o + w]

            mask16 = pka("mask16"); m432 = pka("m432"); m496 = pka("m496")
            m9632 = pka("m9632"); m480 = pka("m480")
            x832 = pka("x832"); x432 = pka("x432"); x480 = pka("x480")
            bdone4 = pka("bdone4"); e16 = pka("e16"); e4 = pka("e4"); e24 = pka("e24")
            g1 = pka("g1"); g2 = pka("g2"); gf = pka("gf")
            wq = pka("wq"); wk = pka("wk"); wv = pka("wv"); wo = pka("wo")
            wup = pka("wup"); wdn = pka("wdown"); wvoc = pka("wvocab")
            vcols = pka("vcols"); e16j = pka("e16j")
            _lo = PACKL_OFFS["cos_t"][0]
            cos_t = packl[:, _lo:_lo + 128].rearrange("p (a b) -> p a b", b=2)
            _lo = PACKL_OFFS["sin_t"][0]
            sin_t = packl[:, _lo:_lo + 128].rearrange("p (a b) -> p a b", b=2)
            id128 = pka("id128")
            ones_m = pka("ones_m"); one11 = pka("one11")
            ones_c = pka("ones_c"); eps_c = pka("eps_c")

            # ---------------- ACT table prewarm ------------------------------
            warm = sb.tile([1, 4], F32, tag="warm")
            nc.scalar.activation(warm[:, 0:1], one11, AF.Exp, bias=eps_c[:1, :])
            nc.scalar.activation(warm[:, 1:2], one11, AF.Sqrt, bias=eps_c[:1, :])

            # ---------------- embedding via one-hot matmuls ------------------
            # u_j[(g,vlo), f] = (x[row] == 8j+vlo);  h_G = sum_j EB_j.T @ u_j
            u_ts = []
            for j in range(3):
                u = sb.tile([128, 512], F32R, tag=f"u{j}")
                nc.vector.tensor_scalar(out=u, in0=xvg, scalar1=vcols[:, j:j + 1],
                                        scalar2=None, op0=ALU.is_equal)
                u_ts.append(u)

            # ---------------- block-diagonal weights -------------------------
            # wkp = diag(g1) @ Wk ; wvp likewise ; wqp = 0.5 * diag(g1) @ Wq
            wkp = sb.tile([D, D], F32, tag="wkp")
            nc.vector.tensor_scalar_mul(wkp, wk, g1)
            wvp = sb.tile([D, D], F32, tag="wvp")
            nc.vector.tensor_scalar_mul(wvp, wv, g1)
            wqp = sb.tile([D, D], F32, tag="wqp")
            nc.vector.tensor_scalar_mul(wqp, wq, g1)
            nc.scalar.mul(wqp, wqp, 0.5)
            wupp = sb.tile([D, Fdim], F32, tag="wupp")
            nc.vector.tensor_scalar_mul(wupp, wup, g2)
            wvocp = sb.tile([D, V], F32, tag="wvocp")
            nc.vector.tensor_scalar_mul(wvocp, wvoc, gf)

            def blockdiag(w_sb, e_mat, mask, P, M, blk_p, blk_f, dtype, tag):
                """[P, M] block-diagonal: rep = E.T @ w ; bd = rep_bcast * mask."""
                rep_ps = psB.tile([P, blk_f], F32, tag="small")
                nc.tensor.matmul(rep_ps, e_mat, w_sb)
                bd = sb.tile([P, M], dtype, tag=tag)
                rep_b = _ap(rep_ps[:, :], [[0, M // blk_f], [1, blk_f]])
                nc.vector.tensor_tensor(out=bd, in0=rep_b, in1=mask, op=ALU.mult)
                return bd

            bdk = blockdiag(wkp, e16, mask16, 128, 128, 8, 8, F32R, "bdk")
            bdv = blockdiag(wvp, e16, mask16, 128, 128, 8, 8, F32R, "bdv")
            bdq4 = blockdiag(wqp, e4, m432, 32, 32, 8, 8, F32, "bdq4")
            bdo4 = blockdiag(wo, e4, m432, 32, 32, 8, 8, F32, "bdo4")
            bdup4 = blockdiag(wupp, e4, m496, 32, 96, 8, 24, F32, "bdup4")
            bddn4 = blockdiag(wdn, e24, m9632, 96, 32, 24, 8, F32, "bddn4")
            bdvoc4 = blockdiag(wvocp, e4, m480, 32, 80, 8, 20, F32, "bdvoc4")
            ebj = [blockdiag(emb24, e16j[:, 128 * j:128 * (j + 1)], mask16,
                             128, 128, 8, 8, F32R, f"eb{j}")
                   for j in range(3)]

            hG_ps = psH.tile([128, 512], F32, tag="hg")
            for j in range(3):
                nc.tensor.matmul(hG_ps, ebj[j], u_ts[j], start=(j == 0), stop=(j == 2))
            h_G = sb.tile([128, 512], F32, tag="h_G")
            nc.vector.tensor_copy(h_G, hG_ps)
            h_Gr = sb.tile([128, 512], F32R, tag="h_Gr")
            nc.scalar.copy(h_Gr, hG_ps)

            # ---------------- h -> R layout (4 transposes) -------------------
            hR_ps = psA.tile([128, 512], F32, tag="big")
            for c in range(4):
                nc.tensor.transpose(hR_ps[:, 128 * c:128 * (c + 1)],
                                    h_G[:, 128 * c:128 * (c + 1)],
                                    id128)
            h_R = sb.tile([128, 64, 8], F32, tag="h_R")
            hRf = h_R.rearrange("p a b -> p (a b)")
            nc.scalar.copy(hRf[:, 0:256], hR_ps[:, 0:256])
            nc.vector.tensor_copy(hRf[:, 256:512], hR_ps[:, 256:512])

            # ---------------- rmsnorm stats ----------------------------------
            hsq = sb.tile([128, 64, 8], F32, tag="hsq")
            nc.gpsimd.tensor_mul(hsq, h_R, h_R)
            ssq = sb.tile([128, 64], F32, tag="ssq")
            nc.vector.reduce_sum(ssq, hsq, axis=AX.X)
            rms = sb.tile([128, 64], F32, tag="rms")
            nc.scalar.activation(rms, ssq, AF.Sqrt, bias=eps_c[:, :], scale=1.0 / D)
            inv1 = sb.tile([128, 64], F32, tag="inv1")
            nc.vector.reciprocal(inv1, rms)

            # cos/sin with inv1 folded (k = rope(kraw) * inv1[row])
            cosI = sb.tile([128, 64, 2], F32, tag="cosI")
            inv1_b = _ap(inv1[:, :], [[1, 64], [0, 2]])
            nc.gpsimd.tensor_tensor(out=cosI, in0=cos_t, in1=inv1_b, op=ALU.mult)
            sinI = sb.tile([128, 64, 2], F32, tag="sinI")
            nc.gpsimd.tensor_tensor(out=sinI, in0=sin_t, in1=inv1_b, op=ALU.mult)

            # ---------------- projections (G layout) -------------------------
            kraw_ps = psA.tile([128, 512], F32, tag="big")
            nc.tensor.matmul(kraw_ps, bdk, h_Gr)
            vraw_ps = psA.tile([128, 512], F32, tag="big")
            nc.tensor.matmul(vraw_ps, bdv, h_Gr)

            kraw_sb = sb.tile([128, 512], F32, tag="kraw_sb")
            nc.scalar.copy(kraw_sb[:, 0:256], kraw_ps[:, 0:256])
            nc.vector.tensor_copy(kraw_sb[:, 256:512], kraw_ps[:, 256:512])
            vraw_sb = sb.tile([128, 512], F32, tag="vraw_sb")
            nc.scalar.copy(vraw_sb[:, 0:256], vraw_ps[:, 0:256])
            nc.vector.tensor_copy(vraw_sb[:, 256:512], vraw_ps[:, 256:512])

            kR_ps = psA.tile([128, 512], F32, tag="big")
            vR_ps = psA.tile([128, 512], F32, tag="big")
            for c in range(4):
                nc.tensor.transpose(kR_ps[:, 128 * c:128 * (c + 1)],
                                    kraw_sb[:, 128 * c:128 * (c + 1)],
                                    id128)
                nc.tensor.transpose(vR_ps[:, 128 * c:128 * (c + 1)],
                                    vraw_sb[:, 128 * c:128 * (c + 1)],
                                    id128)

            # ---------------- rope on k (R layout) ---------------------------
            # k view [128, 64 t, 2 h, 2 j, 2 par]; cosI [128, 64, 2(j)]
            kR_sb = sb.tile([128, 512], F32, tag="kR_sb")
            nc.scalar.copy(kR_sb, kR_ps)
            def kview(tile, par):
                return _ap(tile[:, :], [[8, 64], [4, 2], [2, 2], [1, 1]], extra_off=par)
            cos_b = _ap(cosI[:, :, :], [[2, 64], [0, 2], [1, 2], [0, 1]])
            sin_b = _ap(sinI[:, :, :], [[2, 64], [0, 2], [1, 2], [0, 1]])
            ke = kview(kR_sb, 0); ko = kview(kR_sb, 1)
            t1 = sb.tile([128, 64, 2, 2, 1], F32, tag="t1")
            t2 = sb.tile([128, 64, 2, 2, 1], F32, tag="t2")
            t3 = sb.tile([128, 64, 2, 2, 1], F32, tag="t3")
            t4 = sb.tile([128, 64, 2, 2, 1], F32, tag="t4")
            nc.vector.tensor_tensor(out=t1, in0=ke, in1=cos_b, op=ALU.mult)
            nc.vector.tensor_tensor(out=t2, in0=ko, in1=sin_b, op=ALU.mult)
            nc.vector.tensor_tensor(out=t3, in0=ke, in1=sin_b, op=ALU.mult)
            nc.gpsimd.tensor_tensor(out=t4, in0=ko, in1=cos_b, op=ALU.mult)
            k_R = sb.tile([128, 64, 8], F32, tag="k_R")
            k_e = _ap(k_R[:, :, :], [[8, 64], [4, 2], [2, 2], [1, 1]], extra_off=0)
            k_o = _ap(k_R[:, :, :], [[8, 64], [4, 2], [2, 2], [1, 1]], extra_off=1)
            nc.vector.tensor_tensor(out=k_e, in0=t1, in1=t2, op=ALU.subtract)
            nc.gpsimd.tensor_tensor(out=k_o, in0=t3, in1=t4, op=ALU.add)

            # ---------------- q at position S-1 ------------------------------
            # h_last strip: h_R[0, 51+4b, :]  -> [1, 32] (b, d)
            hl_row = _ap(h_R[0:1, 51, :], [[32, 4], [1, 8]])
            hl_flat = sb.tile([1, 32], F32, tag="hl_flat")
            nc.vector.tensor_copy(hl_flat, hl_row)
            hl_ps = psB.tile([32, 1], F32, tag="small")
            nc.tensor.matmul(hl_ps, hl_flat, one11)
            hl_sb = sb.tile([32, 1], F32, tag="hl_sb")
            nc.scalar.copy(hl_sb, hl_ps)
            qc_ps = psB.tile([32, 1], F32, tag="small")
            nc.tensor.matmul(qc_ps, bdq4, hl_sb)
            qc_sb = sb.tile([32, 1], F32, tag="qc_sb")
            nc.scalar.copy(qc_sb, qc_ps)
            qr_ps = psB.tile([1, 32], F32, tag="small")
            nc.tensor.matmul(qr_ps, qc_sb, id128[:32, :32])
            q_row = sb.tile([1, 32], F32, tag="q_row")
            nc.scalar.copy(q_row, qr_ps)
            # scale by inv1 of the last-token rows: inv1[0, 51+4b]
            inv1_strip = _ap(inv1[0:1, 51:52], [[4, 4], [0, 8]])
            qs = sb.tile([1, 32], F32, tag="qs")
            nc.vector.tensor_tensor(out=qs, in0=_ap(q_row[:, :], [[8, 4], [1, 8]]),
                                    in1=inv1_strip, op=ALU.mult)
            # rope at s = 2047: cos/sin from tables at [0, 51, j]
            cq = _ap(cos_t[0:1, 51, :], [[0, 4], [0, 2], [1, 2]])
            sq = _ap(sin_t[0:1, 51, :], [[0, 4], [0, 2], [1, 2]])
            def qview(tile, par):
                return _ap(tile[:, :], [[8, 4], [4, 2], [2, 2]], extra_off=par)
            qe = qview(qs, 0); qo = qview(qs, 1)
            u1 = sb.tile([1, 16], F32, tag="u1"); u2 = sb.tile([1, 16], F32, tag="u2")
            u3 = sb.tile([1, 16], F32, tag="u3"); u4 = sb.tile([1, 16], F32, tag="u4")
            nc.vector.tensor_tensor(out=u1, in0=qe, in1=cq, op=ALU.mult)
            nc.vector.tensor_tensor(out=u2, in0=qo, in1=sq, op=ALU.mult)
            nc.vector.tensor_tensor(out=u3, in0=qe, in1=sq, op=ALU.mult)
            nc.vector.tensor_tensor(out=u4, in0=qo, in1=cq, op=ALU.mult)
            qfin = sb.tile([1, 32], F32, tag="qfin")
            qf_e = qview(qfin, 0); qf_o = qview(qfin, 1)
            u1v = _ap(u1[:, :], [[4, 4], [2, 2], [1, 2]])
            u2v = _ap(u2[:, :], [[4, 4], [2, 2], [1, 2]])
            u3v = _ap(u3[:, :], [[4, 4], [2, 2], [1, 2]])
            u4v = _ap(u4[:, :], [[4, 4], [2, 2], [1, 2]])
            nc.vector.tensor_tensor(out=qf_e, in0=u1v, in1=u2v, op=ALU.subtract)
            nc.vector.tensor_tensor(out=qf_o, in0=u3v, in1=u4v, op=ALU.add)

            # replicate q over all rows: qrep[p, t, d] = qfin[b(t), hhd]
            onesr = sb.tile([1, 128], F32R, tag="onesr")
            nc.scalar.copy(onesr, ones_m)
            qext = sb.tile([1, 128], F32R, tag="qext")
            nc.vector.tensor_copy(qext, _ap(qfin[:, :], [[8, 4], [0, 4], [1, 8]]))
            qrep_ps = psA.tile([128, 512], F32, tag="big")
            qsrc = _ap(qext[:, :], [[0, 4], [1, 128]])
            nc.tensor.matmul(qrep_ps, onesr, qsrc)

            # ---------------- scores + softmax (no max-sub; scores bounded) --
            sprod = sb.tile([128, 64, 2, 4], F32, tag="sprod")
            nc.vector.tensor_tensor(out=sprod, in0=_ap(k_R[:, :, :], [[8, 64], [4, 2], [1, 4]]),
                                    in1=_ap(qrep_ps[:, :], [[8, 64], [4, 2], [1, 4]]),
                                    op=ALU.mult)
            scores = sb.tile([128, 64, 2], F32, tag="scores")
            nc.vector.reduce_sum(scores, sprod, axis=AX.X)
            exp_t = sb.tile([128, 64, 2], F32, tag="exp_t")
            nc.scalar.activation(exp_t, scores, AF.Exp)
            expinv = sb.tile([128, 64, 2], F32, tag="expinv")
            nc.vector.tensor_tensor(out=expinv, in0=exp_t, in1=inv1_b, op=ALU.mult)
            vw = sb.tile([128, 64, 2, 4], F32, tag="vw")
            nc.vector.tensor_tensor(out=vw, in0=_ap(vR_ps[:, :], [[8, 64], [4, 2], [1, 4]]),
                                    in1=_ap(expinv[:, :, :], [[2, 64], [1, 2], [0, 4]]),
                                    op=ALU.mult)

            # per-batch partial reductions -> numden [128, 40]
            numden = sb.tile([128, 40], F32, tag="numden")
            for b in range(B):
                nin = _ap(vw[:, :, :, :], [[1, 8], [128, 4], [8, 4]], extra_off=32 * b)
                nc.vector.reduce_sum(numden[:, 8 * b:8 * (b + 1)], nin, axis=AX.XY)
            for b in range(B):
                din_ = _ap(exp_t[:, :, :], [[1, 2], [32, 4], [2, 4]], extra_off=8 * b)
                nc.vector.reduce_sum(numden[:, 32 + 2 * b:34 + 2 * b], din_, axis=AX.XY)

            combo_ps = psB.tile([40, 1], F32, tag="small")
            nc.tensor.matmul(combo_ps, numden, ones_c)

            # ---------------- tail: ctx, attn-out, FFN, final norm ----------
            den_sb = sb.tile([8, 1], F32, tag="den_sb")
            nc.scalar.copy(den_sb, combo_ps[32:40, :])
            rden = sb.tile([8, 1], F32, tag="rden")
            nc.vector.reciprocal(rden, den_sb)
            rdx_ps = psB.tile([32, 1], F32, tag="small")
            nc.tensor.matmul(rdx_ps, x832, rden)
            num_sb = sb.tile([32, 1], F32, tag="num_sb")
            nc.scalar.copy(num_sb, combo_ps[0:32, :])
            ctx = sb.tile([32, 1], F32, tag="ctx")
            nc.vector.tensor_tensor(out=ctx, in0=num_sb, in1=rdx_ps, op=ALU.mult)

            h2_ps = psB.tile([32, 1], F32, tag="small")
            nc.tensor.matmul(h2_ps, hl_flat, one11, start=True, stop=False)
            nc.tensor.matmul(h2_ps, bdo4, ctx, start=False, stop=True)
            h2_sb = sb.tile([32, 1], F32, tag="h2_sb")
            nc.scalar.copy(h2_sb, h2_ps)

            h2sq = sb.tile([32, 1], F32, tag="h2sq")
            nc.vector.tensor_mul(h2sq, h2_sb, h2_sb)
            ssq2_ps = psB.tile([4, 1], F32, tag="small")
            nc.tensor.matmul(ssq2_ps, bdone4, h2sq)
            rms2 = sb.tile([4, 1], F32, tag="rms2")
            nc.scalar.activation(rms2, ssq2_ps, AF.Sqrt, bias=eps_c[:4, :], scale=1.0 / D)
            inv2 = sb.tile([4, 1], F32, tag="inv2")
            nc.vector.reciprocal(inv2, rms2)

            y_ps = psB.tile([96, 1], F32, tag="small")
            nc.tensor.matmul(y_ps, bdup4, h2_sb)
            frelu = sb.tile([96, 1], F32, tag="frelu")
            nc.vector.tensor_scalar(out=frelu, in0=y_ps, scalar1=0.0, scalar2=None, op0=ALU.max)
            dl_ps = psB.tile([32, 1], F32, tag="small")
            nc.tensor.matmul(dl_ps, bddn4, frelu)
            i2x_ps = psB.tile([32, 1], F32, tag="small")
            nc.tensor.matmul(i2x_ps, x432, inv2)
            i2x_sb = sb.tile([32, 1], F32, tag="i2x_sb")
            nc.scalar.copy(i2x_sb, i2x_ps)
            h3 = sb.tile([32, 1], F32, tag="h3")
            nc.vector.scalar_tensor_tensor(out=h3, in0=dl_ps, scalar=i2x_sb,
                                           in1=h2_sb, op0=ALU.mult, op1=ALU.add)

            h3sq = sb.tile([32, 1], F32, tag="h3sq")
            nc.vector.tensor_mul(h3sq, h3, h3)
            ssq3_ps = psB.tile([4, 1], F32, tag="small")
            nc.tensor.matmul(ssq3_ps, bdone4, h3sq)
            rms3 = sb.tile([4, 1], F32, tag="rms3")
            nc.scalar.activation(rms3, ssq3_ps, AF.Sqrt, bias=eps_c[:4, :], scale=1.0 / D)
            inv3 = sb.tile([4, 1], F32, tag="inv3")
            nc.vector.reciprocal(inv3, rms3)

            lr_ps = psB.tile([80, 1], F32, tag="small")
            nc.tensor.matmul(lr_ps, bdvoc4, h3)
            i3x_ps = psB.tile([80, 1], F32, tag="small")
            nc.tensor.matmul(i3x_ps, x480, inv3)
            i3x_sb = sb.tile([80, 1], F32, tag="i3x_sb")
            nc.scalar.copy(i3x_sb, i3x_ps)
            logits_sb = sb.tile([80, 1], F32, tag="logits_sb")
            nc.vector.tensor_tensor(out=logits_sb, in0=lr_ps, in1=i3x_sb, op=ALU.mult)
            nc.sync.dma_start(out=out_d.rearrange("(a b) -> a b", b=1), in_=logits_sb)

            if debug:
                local = locals()
                for nm in dbg_out:
                    t = local[nm]
                    flat = bass.AP(tensor=t.tensor, offset=t.offset,
                                   ap=[list(t.ap[0]), [1, t.free_size()]])
                    nc.sync.dma_start(out=dbg_out[nm], in_=flat)

    nc.finalize()
    return nc


_CACHE = {}


def _in_map(x, emb, g1, Wq, Wk, Wv, Wo, g2, Wup, Wdown, gf, Wvocab):
    packa, packl, emb24 = _build_packs(emb, g1, g2, gf, Wq, Wk, Wv, Wo, Wup, Wdown, Wvocab)
    return {"xvg": _pack_x(x), "packa": packa, "packl": packl, "emb24": emb24}


def kernel(x, emb, g1, Wq, Wk, Wv, Wo, g2, Wup, Wdown, gf, Wvocab):
    if "nc" not in _CACHE:
        _CACHE["nc"] = build_nc()
    nc = _CACHE["nc"]
    m = _in_map(x, emb, g1, Wq, Wk, Wv, Wo, g2, Wup, Wdown, gf, Wvocab)
    in_maps = [m for _ in range(NCORES)]
    res = run_bass_kernel_spmd(nc, in_maps, core_ids=list(range(NCORES)))
    logits = np.asarray(res.results[0]["logits"], dtype=np.float32)
    return logits.reshape(B, V)
